# revision 1
# baseline (speedup 1.0000x reference)
"""Trainium2 Bass kernel for an MoE routing module.

Strategy: data-parallel over the batch — each of the 8 NeuronCores runs the
full pipeline (gating -> top-2 -> expert MLPs) for its 8 samples. All
data-dependent expert selection is done with indirect-DMA gathers driven by
index tiles computed on device; there are no collectives and no registers.

Host-side prep is limited to dtype casts and weight re-layouts:
  - expert weights are packed into ONE bf16 "mega table" [E*128, 8336] so a
    single [128,1] index tile (value e*128+p) gathers W1+W2+b1+b2 for an
    expert in one indirect DMA with 128 fat descriptors:
      cols 0..8191   W1[e, t*128+p, h]   (t-major)
      cols 8192..8319 W2[e, j*128+p, c]  (j-major)
      cols 8320..8327 b1[e, t*128+p]
      col  8328       b2[e, p] (valid on partitions 0..15)
  - gating tokens are gathered with dma_gather (int16 vocab indices,
    pre-wrapped on host into the [16-partition x replicated-across-cores]
    layout the Q7 ucode expects).
Expert math is bf16 (fp32 PSUM); the gating path is fp32 so top-2 selection
matches the fp32 reference. Samples are processed in 2 groups of 4 so expert
compute of group 0 overlaps gating of group 1.

HW gotcha (verified on device): indirect DMA consumes exactly ONE index per
destination partition — multi-index-per-partition gathers return garbage.
"""

import os
import sys

for _p in ("/opt/trn_rl_repo", "/root/.axon_site/_ro/trn_rl_repo"):
    if os.path.isdir(_p) and _p not in sys.path:
        sys.path.insert(0, _p)

import numpy as np

import concourse.bacc as bacc
import concourse.tile as tile
import concourse.mybir as mybir
from concourse.bass import IndirectOffsetOnAxis
from concourse.bass_utils import run_bass_kernel_spmd
from concourse.masks import make_identity

F32 = mybir.dt.float32
BF16 = mybir.dt.bfloat16
I32 = mybir.dt.int32
I16 = mybir.dt.int16
U32 = mybir.dt.uint32

V, D, H, E, C, TOPK = 16000, 1024, 1024, 8, 16, 2
B, S = 64, 512
GATE_H = 256
NCORES = 8
BL = B // NCORES          # samples per core
DT = D // 128             # 8 d-tiles
HT = H // 128             # 8 h-tiles
ST = S // 128             # 4 s-tiles
MT = GATE_H // 128        # 2 gate-hidden tiles
NGRP = 2                  # sample groups per core (pipelining)
GBL = BL // NGRP          # samples per group

# mega weight table columns (W2 stored as bf16 hi+lo so it reconstructs to
# ~fp32 on device — bf16-quantized W2 alone costs 1.7e-3 rel err)
W1COL = 0
W2COL = DT * H            # 8192  (hi)
W2LO = W2COL + HT * C     # 8320  (lo)
B1COL = W2LO + HT * C     # 8448
B2COL = B1COL + HT        # 8456
WCOLS = 8464              # padded row length

_compiled = {}
last_results = None       # BassKernelResults of the most recent run (for test.py)


def build_program(reps=1):
    """reps>1 repeats the whole compute body (benchmarking aid)."""
    nc = bacc.Bacc("TRN2", target_bir_lowering=False, debug=False, num_devices=NCORES)
    act = mybir.ActivationFunctionType

    x_t = nc.dram_tensor("x_loc", [BL, S], I32, kind="ExternalInput")
    xw_t = nc.dram_tensor("xw16", [128, BL, S // 16], I16, kind="ExternalInput")
    emb_t = nc.dram_tensor("emb", [V, D], F32, kind="ExternalInput")
    eemb_t = nc.dram_tensor("eemb", [E * V, D], BF16, kind="ExternalInput")
    wall_t = nc.dram_tensor("wall", [E * 128, WCOLS], BF16, kind="ExternalInput")
    gw1_t = nc.dram_tensor("gw1", [D, GATE_H], F32, kind="ExternalInput")
    gb1_t = nc.dram_tensor("gb1", [128, MT], F32, kind="ExternalInput")
    gw2_t = nc.dram_tensor("gw2", [GATE_H, E], F32, kind="ExternalInput")
    gb2_t = nc.dram_tensor("gb2", [E, 1], F32, kind="ExternalInput")
    out_t = nc.dram_tensor("out", [BL, C], F32, kind="ExternalOutput")

    with tile.TileContext(nc) as tc:
        with (
            tc.tile_pool(name="const", bufs=1) as cpool,
            tc.tile_pool(name="dram", bufs=1, space="DRAM") as dpool,
        ):
            # ---- constants ----
            id_bf = cpool.tile([128, 128], BF16)
            make_identity(nc, id_bf[:, :])
            id_f = cpool.tile([128, 128], F32)
            make_identity(nc, id_f[:, :])
            ones_k = cpool.tile([128, 1], F32)      # lhsT for partition-sum MMs
            nc.vector.memset(ones_k[:, :], 1.0)
            ones_m = cpool.tile([1, 128], F32)      # lhsT for K=1 broadcast MMs
            nc.vector.memset(ones_m[:, :], 1.0)
            iota_p = cpool.tile([128, 1], I32)      # value = partition index
            nc.gpsimd.iota(iota_p[:, :], pattern=[[0, 1]], base=0, channel_multiplier=1)

            # token ids, transposed: xt[p, b, t] = x[b, t*128+p]
            xt = cpool.tile([128, BL, ST], I32)
            nc.sync.dma_start(
                out=xt[:, :, :], in_=x_t[:, :].rearrange("b (t p) -> p b t", p=128)
            )
            # int16 wrapped indices for dma_gather (pre-wrapped on host)
            xw = cpool.tile([128, BL, S // 16], I16)
            nc.sync.dma_start(out=xw[:, :, :], in_=xw_t[:, :, :])

            gb1_sb = cpool.tile([128, MT], F32)
            nc.sync.dma_start(out=gb1_sb[:, :], in_=gb1_t[:, :])
            gb2_sb = cpool.tile([E, 1], F32)
            nc.sync.dma_start(out=gb2_sb[:, :], in_=gb2_t[:, :])
            gw1_sb = cpool.tile([128, DT, GATE_H], F32)
            nc.sync.dma_start(
                out=gw1_sb[:, :, :], in_=gw1_t[:, :].rearrange("(j p) g -> p j g", p=128)
            )
            gw2_sb = cpool.tile([128, MT, E], F32)
            nc.sync.dma_start(
                out=gw2_sb[:, :, :], in_=gw2_t[:, :].rearrange("(m p) e -> p m e", p=128)
            )

            consts = dict(
                id_bf=id_bf, id_f=id_f, ones_k=ones_k, ones_m=ones_m,
                iota_p=iota_p, xt=xt, xw=xw, gb1_sb=gb1_sb, gb2_sb=gb2_sb,
                gw1_sb=gw1_sb, gw2_sb=gw2_sb,
            )
            tensors = dict(
                emb_t=emb_t, eemb_t=eemb_t, wall_t=wall_t, out_t=out_t,
            )
            # chain tile serializes reps so the benchmark differential is honest
            chain = None
            if reps > 1:
                chain = cpool.tile([1, 1], F32)
                nc.vector.memset(chain[:, :], 0.0)
            for rep in range(reps):
                _body_once(nc, tc, act, rep, dpool, consts, tensors, chain)

    nc.compile()
    return nc


def _body_once(nc, tc, act, rep, dpool, cn, tn, chain=None):
    sfx = f"_r{rep}"
    id_bf, id_f = cn["id_bf"], cn["id_f"]
    ones_k, ones_m, iota_p = cn["ones_k"], cn["ones_m"], cn["iota_p"]
    xt, xw = cn["xt"], cn["xw"]
    gb1_sb, gb2_sb, gw1_sb, gw2_sb = cn["gb1_sb"], cn["gb2_sb"], cn["gw1_sb"], cn["gw2_sb"]
    emb_t, eemb_t, wall_t, out_t = tn["emb_t"], tn["eemb_t"], tn["wall_t"], tn["out_t"]

    with (
        tc.tile_pool(name=f"persist{sfx}", bufs=1) as ppool,
        tc.tile_pool(name=f"bc{sfx}", bufs=2) as bcpool,
        # gating pools
        tc.tile_pool(name=f"gat{sfx}", bufs=2) as gpool,
        tc.tile_pool(name=f"gat1{sfx}", bufs=2) as g1pool,
        tc.tile_pool(name=f"gsb{sfx}", bufs=2) as gspool,
        tc.tile_pool(name=f"gps{sfx}", bufs=1, space="PSUM") as gps,
        tc.tile_pool(name=f"gpss{sfx}", bufs=2, space="PSUM") as gps_s,
        # expert pools
        tc.tile_pool(name=f"exi{sfx}", bufs=3) as xipool,
        tc.tile_pool(name=f"etok{sfx}", bufs=2) as tokpool,
        tc.tile_pool(name=f"ew{sfx}", bufs=2) as wpool,
        tc.tile_pool(name=f"ett{sfx}", bufs=2) as ttpool,
        tc.tile_pool(name=f"esm{sfx}", bufs=3) as smpool,
        tc.tile_pool(name=f"ejunk{sfx}", bufs=2) as junkpool,
        tc.tile_pool(name=f"epst{sfx}", bufs=2, space="PSUM") as eps_t,
        tc.tile_pool(name=f"epsz{sfx}", bufs=2, space="PSUM") as eps_z,
        tc.tile_pool(name=f"epso{sfx}", bufs=1, space="PSUM") as eps_o,
    ):
        out_acc = ppool.tile([C, BL], F32)
        nc.vector.memset(out_acc[:, :], 0.0)

        for g in range(NGRP):
            b0 = g * GBL
            # ============ gating for samples [b0, b0+GBL) (fp32) ============
            pooled = gspool.tile([1, GBL * D], F32, tag="pooled")
            for bl in range(GBL):
                b = b0 + bl
                gtok = gpool.tile([128, ST, D], F32, tag="gtok")
                nc.gpsimd.dma_gather(
                    out_ap=gtok[:, :, :],
                    in_ap=emb_t[:, :],
                    idxs_ap=xw[:, b, :],
                    num_idxs=S,
                    num_idxs_reg=S,
                    elem_size=D,
                    transpose=False,
                )
                t01 = g1pool.tile([128, D], F32, tag="t01")
                t23 = g1pool.tile([128, D], F32, tag="t23")
                nc.vector.tensor_add(t01[:, :], gtok[:, 0, :], gtok[:, 1, :])
                nc.vector.tensor_add(t23[:, :], gtok[:, 2, :], gtok[:, 3, :])
                cb = g1pool.tile([128, D], F32, tag="cb")
                nc.vector.tensor_add(cb[:, :], t01[:, :], t23[:, :])
                for h in range(2):
                    pp = gps.tile([1, 512], F32, tag="pool_ps")
                    nc.tensor.matmul(
                        out=pp[:, :],
                        lhsT=ones_k[:, :],
                        rhs=cb[:, h * 512 : (h + 1) * 512],
                        start=True,
                        stop=True,
                    )
                    # mean over S
                    nc.scalar.activation(
                        out=pooled[0:1, bl * D + h * 512 : bl * D + (h + 1) * 512],
                        in_=pp[:, :],
                        func=act.Copy,
                        scale=1.0 / S,
                    )

            # pooled^T [d, b] via K=1 matmuls into one psum tile
            pt_ps = gps_s.tile([128, DT * GBL], F32, tag="gmisc")
            for bl in range(GBL):
                for j in range(DT):
                    nc.tensor.matmul(
                        out=pt_ps[:, j * GBL + bl : j * GBL + bl + 1],
                        lhsT=pooled[0:1, bl * D + j * 128 : bl * D + (j + 1) * 128],
                        rhs=ones_m[0:1, 0:1],
                        start=True,
                        stop=True,
                    )
            pts = gspool.tile([128, DT * GBL], F32, tag="pts")
            nc.vector.tensor_copy(pts[:, :], pt_ps[:, :])

            # gate layer 1 + relu
            hR = gspool.tile([128, MT, GBL], F32, tag="hR")
            for m in range(MT):
                h_ps = gps_s.tile([128, GBL], F32, tag="gmisc")
                for j in range(DT):
                    nc.tensor.matmul(
                        out=h_ps[:, :],
                        lhsT=gw1_sb[:, j, m * 128 : (m + 1) * 128],
                        rhs=pts[:, j * GBL : (j + 1) * GBL],
                        start=(j == 0),
                        stop=(j == DT - 1),
                    )
                nc.scalar.activation(
                    out=hR[:, m, :],
                    in_=h_ps[:, :],
                    func=act.Relu,
                    bias=gb1_sb[:, m : m + 1],
                )

            # gate layer 2 -> logits [e, b]
            l_ps = gps_s.tile([E, GBL], F32, tag="gmisc")
            for m in range(MT):
                nc.tensor.matmul(
                    out=l_ps[:, :],
                    lhsT=gw2_sb[:, m, :],
                    rhs=hR[:, m, :],
                    start=(m == 0),
                    stop=(m == MT - 1),
                )
            l_sb = gspool.tile([E, GBL], F32, tag="l_sb")
            nc.scalar.activation(
                out=l_sb[:, :], in_=l_ps[:, :], func=act.Identity,
                bias=gb2_sb[:, 0:1],
            )
            # transpose logits -> [b, e]
            lt_ps = gps_s.tile([GBL, E], F32, tag="gmisc")
            nc.tensor.matmul(
                out=lt_ps[:, :], lhsT=l_sb[:, :], rhs=id_f[0:E, 0:E],
                start=True, stop=True,
            )
            lt_sb = gspool.tile([GBL, E], F32, tag="lt_sb")
            nc.vector.tensor_copy(lt_sb[:, :], lt_ps[:, :])

            # top-2 of logits == top-2 of softmax (monotone)
            mx = gspool.tile([GBL, 8], F32, tag="mx")
            mi = gspool.tile([GBL, 8], U32, tag="mi")
            nc.vector.max_with_indices(mx[:, :], mi[:, :], lt_sb[:, :])

            # renormalized top-2 softmax weights:
            # rw1 = 1/(1+exp(l2-l1)), rw2 = exp(l2-l1)/(1+exp(l2-l1))
            dlt = gspool.tile([GBL, 1], F32, tag="dlt")
            nc.vector.tensor_sub(dlt[:, :], mx[:, 1:2], mx[:, 0:1])
            q = gspool.tile([GBL, 1], F32, tag="q")
            nc.scalar.activation(out=q[:, :], in_=dlt[:, :], func=act.Exp)
            sden = gspool.tile([GBL, 1], F32, tag="sden")
            nc.vector.tensor_scalar_add(sden[:, :], q[:, :], 1.0)
            rw1 = gspool.tile([GBL, 1], F32, tag="rw1")
            nc.vector.reciprocal(rw1[:, :], sden[:, :])
            rw2 = gspool.tile([GBL, 1], F32, tag="rw2")
            nc.vector.tensor_mul(rw2[:, :], q[:, :], rw1[:, :])

            # pack per-(b,k) scalars: cols bl*8 + {0,1}=e*V, {2,3}=e*128,
            # {6,7}=rw ({4,5} unused)
            ei_f = gspool.tile([GBL, TOPK], F32, tag="ei_f")
            nc.vector.tensor_copy(ei_f[:, :], mi[:, 0:TOPK])
            vals = gspool.tile([GBL, 8], F32, tag="vals")
            nc.vector.tensor_scalar_mul(vals[:, 0:2], ei_f[:, :], float(V))
            nc.vector.tensor_scalar_mul(vals[:, 2:4], ei_f[:, :], 128.0)
            nc.vector.tensor_scalar_mul(vals[:, 4:6], ei_f[:, :], 0.0)
            nc.vector.tensor_copy(vals[:, 6:7], rw1[:, :])
            nc.vector.tensor_copy(vals[:, 7:8], rw2[:, :])

            # broadcast across partitions: bounce through DRAM to get a flat
            # [1, GBL*8] row, then K=1 matmul against ones.
            scratch = dpool.tile([GBL, 8], F32, tag=f"scratch{sfx}_{g}")
            nc.sync.dma_start(out=scratch[:, :], in_=vals[:, :])
            if chain is not None:
                # unused col 4: forces rep r to wait on rep r-1's result
                nc.sync.dma_start(out=scratch[0:1, 4:5], in_=chain[0:1, 0:1])
            flat = gspool.tile([1, GBL * 8], F32, tag="flat")
            nc.sync.dma_start(
                out=flat[0:1, :].rearrange("p (b c) -> p b c", b=GBL),
                in_=scratch[:, :],
            )
            bc_ps = gps_s.tile([128, GBL * 8], F32, tag="gmisc")
            nc.tensor.matmul(
                out=bc_ps[:, :], lhsT=ones_m[:, :], rhs=flat[0:1, :],
                start=True, stop=True,
            )
            BCf = bcpool.tile([128, GBL * 8], F32, tag="bcf")
            BCi = bcpool.tile([128, GBL * 8], I32, tag="bci")
            nc.vector.tensor_copy(BCf[:, :], bc_ps[:, :])
            nc.vector.tensor_copy(BCi[:, :], bc_ps[:, :])  # cast f32->i32

            # ============ experts for this group (bf16) ============
            for bl in range(GBL):
                b = b0 + bl
                for k in range(TOPK):
                    cEV = bl * 8 + k
                    cE128 = bl * 8 + 2 + k
                    cRW = bl * 8 + 6 + k

                    tok_idx = xipool.tile([128, ST], I32, tag="tok_idx")
                    nc.vector.tensor_add(
                        tok_idx[:, :],
                        xt[:, b, :],
                        BCi[:, cEV : cEV + 1].to_broadcast([128, ST]),
                    )
                    w_idx = xipool.tile([128, 1], I32, tag="w_idx")
                    nc.vector.tensor_add(
                        w_idx[:, :], iota_p[:, :], BCi[:, cE128 : cE128 + 1]
                    )

                    tok = tokpool.tile([128, ST, D], BF16, tag="tok")
                    for t in range(ST):
                        nc.gpsimd.indirect_dma_start(
                            out=tok[:, t, :],
                            out_offset=None,
                            in_=eemb_t[:, :],
                            in_offset=IndirectOffsetOnAxis(
                                ap=tok_idx[:, t : t + 1], axis=0
                            ),
                        )
                    # one gather for W1 + W2 + b1 + b2
                    wg = wpool.tile([128, WCOLS], BF16, tag="wg")
                    nc.gpsimd.indirect_dma_start(
                        out=wg[:, :],
                        out_offset=None,
                        in_=wall_t[:, :],
                        in_offset=IndirectOffsetOnAxis(ap=w_idx[:, :], axis=0),
                    )
                    b1f = smpool.tile([128, HT], F32, tag="b1f")
                    nc.vector.tensor_copy(b1f[:, :], wg[:, B1COL : B1COL + HT])
                    b2f = smpool.tile([C, 1], F32, tag="b2f")
                    nc.vector.tensor_copy(b2f[:, :], wg[0:C, B2COL : B2COL + 1])
                    w2f = smpool.tile([128, HT * C], F32, tag="w2f")
                    nc.vector.tensor_add(
                        w2f[:, :], wg[:, W2COL : W2COL + HT * C],
                        wg[:, W2LO : W2LO + HT * C],
                    )

                    # transpose tok -> tokT[d, s] via matmul against identity
                    tokT = ttpool.tile([128, DT, S], BF16, tag="tokT")
                    for j in range(DT):
                        tp = eps_t.tile([128, S], F32, tag="tp")
                        for t in range(ST):
                            nc.tensor.matmul(
                                out=tp[:, t * 128 : (t + 1) * 128],
                                lhsT=tok[:, t, j * 128 : (j + 1) * 128],
                                rhs=id_bf[:, :],
                                start=True,
                                stop=True,
                            )
                        nc.vector.tensor_copy(tokT[:, j, :], tp[:, :])

                    # z[h_tile] = relu(tokT.T @ W1 + b1); accumulate sum over s
                    pacc = smpool.tile([128, HT], F32, tag="pacc")
                    for j2 in range(HT):
                        z_ps = eps_z.tile([128, S], F32, tag="z")
                        for t in range(DT):
                            nc.tensor.matmul(
                                out=z_ps[:, :],
                                lhsT=wg[:, t * H + j2 * 128 : t * H + (j2 + 1) * 128],
                                rhs=tokT[:, t, :],
                                start=(t == 0),
                                stop=(t == DT - 1),
                            )
                        zjunk = junkpool.tile([128, S], BF16, tag="zjunk")
                        nc.scalar.activation(
                            out=zjunk[:, :],
                            in_=z_ps[:, :],
                            func=act.Relu,
                            bias=b1f[:, j2 : j2 + 1],
                            accum_out=pacc[:, j2 : j2 + 1],
                        )

                    psc = smpool.tile([128, HT], F32, tag="psc")
                    nc.vector.tensor_scalar_mul(psc[:, :], pacc[:, :], 1.0 / S)

                    eo_ps = eps_o.tile([C, 1], F32, tag="eo")
                    for j2 in range(HT):
                        nc.tensor.matmul(
                            out=eo_ps[:, :],
                            lhsT=w2f[:, j2 * C : (j2 + 1) * C],
                            rhs=psc[:, j2 : j2 + 1],
                            start=(j2 == 0),
                            stop=(j2 == HT - 1),
                        )
                    eo1 = smpool.tile([C, 1], F32, tag="eo1")
                    nc.scalar.activation(
                        out=eo1[:, :], in_=eo_ps[:, :], func=act.Identity,
                        bias=b2f[:, 0:1],
                    )
                    eo2 = smpool.tile([C, 1], F32, tag="eo2")
                    nc.vector.tensor_mul(eo2[:, :], eo1[:, :], BCf[0:C, cRW : cRW + 1])
                    nc.vector.tensor_add(
                        out_acc[:, b : b + 1], out_acc[:, b : b + 1], eo2[:, :]
                    )

        if chain is not None:
            nc.vector.tensor_copy(chain[0:1, 0:1], out_acc[0:1, 0:1])
        nc.sync.dma_start(
            out=out_t[:, :].rearrange("b c -> c b"), in_=out_acc[:, :]
        )


def _prep_inputs(inputs):
    """Host-side dtype casts + re-layouts shared by all cores."""
    import ml_dtypes

    f32 = np.float32
    bf16 = ml_dtypes.bfloat16

    x = np.asarray(inputs["x"]).astype(np.int32)
    # int16 indices wrapped for dma_gather: xw16[16g+p, b, c] = x[b, c*16+p]
    xw = x.reshape(B, S // 16, 16).transpose(2, 0, 1).astype(np.int16)  # [16, B, 32]
    xw16 = np.tile(xw, (8, 1, 1))                                       # [128, B, 32]

    emb = np.asarray(inputs["emb"], dtype=f32)
    exp_emb = np.ascontiguousarray(
        np.asarray(inputs["exp_emb"], dtype=f32).reshape(E * V, D)
    ).astype(bf16)

    w1 = np.asarray(inputs["exp_w1"], dtype=f32)          # [E, D, H]
    ew1 = w1.reshape(E, DT, 128, H).transpose(0, 2, 1, 3).reshape(E * 128, DT * H)
    w2 = np.asarray(inputs["exp_w2"], dtype=f32)          # [E, H, C]
    ew2 = w2.reshape(E, HT, 128, C).transpose(0, 2, 1, 3).reshape(E * 128, HT * C)
    b1 = np.asarray(inputs["exp_b1"], dtype=f32)          # [E, H]
    b1r = b1.reshape(E, HT, 128).transpose(0, 2, 1).reshape(E * 128, HT)
    b2 = np.asarray(inputs["exp_b2"], dtype=f32)          # [E, C]
    b2slot = np.zeros((E * 128, 1), f32)
    for e in range(E):
        b2slot[e * 128 : e * 128 + C, 0] = b2[e]
    w2hi = ew2.astype(bf16).astype(f32)
    w2lo = ew2 - w2hi
    wall = np.zeros((E * 128, WCOLS), f32)
    wall[:, W1COL : W1COL + DT * H] = ew1
    wall[:, W2COL : W2COL + HT * C] = w2hi
    wall[:, W2LO : W2LO + HT * C] = w2lo
    wall[:, B1COL : B1COL + HT] = b1r
    wall[:, B2COL : B2COL + 1] = b2slot
    wall = np.ascontiguousarray(wall).astype(bf16)

    gw1 = np.ascontiguousarray(np.asarray(inputs["gate_w1"], dtype=f32))
    gb1 = np.ascontiguousarray(
        np.asarray(inputs["gate_b1"], dtype=f32).reshape(MT, 128).T
    )
    gw2 = np.ascontiguousarray(np.asarray(inputs["gate_w2"], dtype=f32))
    gb2 = np.ascontiguousarray(np.asarray(inputs["gate_b2"], dtype=f32).reshape(E, 1))

    shared = dict(
        emb=emb, eemb=exp_emb, wall=wall,
        gw1=gw1, gb1=gb1, gw2=gw2, gb2=gb2,
    )
    return x, xw16, shared


def kernel(**inputs) -> np.ndarray:
    global last_results
    if "nc" not in _compiled:
        _compiled["nc"] = build_program()
    nc = _compiled["nc"]

    x, xw16, shared = _prep_inputs(inputs)
    in_maps = [
        {
            "x_loc": np.ascontiguousarray(x[c * BL : (c + 1) * BL]),
            "xw16": np.ascontiguousarray(xw16[:, c * BL : (c + 1) * BL]),
            **shared,
        }
        for c in range(NCORES)
    ]
    res = run_bass_kernel_spmd(nc, in_maps, list(range(NCORES)))
    last_results = res
    out = np.concatenate([res.results[c]["out"] for c in range(NCORES)], axis=0)
    return np.ascontiguousarray(out.astype(np.float32))



# revision 4
# speedup vs baseline: 1.4487x; 1.4487x over previous
"""Trainium2 Bass kernel for an MoE routing module.

Strategy: data-parallel over the batch — each of the 8 NeuronCores runs the
full pipeline (gating -> top-2 -> expert MLPs) for its 8 samples. All
data-dependent expert selection is done with indirect-DMA gathers driven by
index tiles computed on device; there are no collectives and no registers.

v2: the expert path runs in FP8 (e4m3, x32 scaling) with DoubleRow matmuls
(2 k-subtiles per instruction at 0.5 cyc/row) and fp8 PE transposes whose
PSUM results are copied out as bitcast u32 (4x fewer DVE elements). The
gating embedding gather is fp16 (top-2 selection margin vs fp32 reference
verified 30x above the fp16 quantization error on the seed-0 inputs); all
gating arithmetic after the gather stays fp32.

Host-side prep is limited to dtype casts and weight re-layouts:
  - exp_emb -> one fp8 table [E*V, D] scaled by 32.
  - exp_w1  -> fp8 "wall8" [E*128, DT*H] scaled by 32, t-major cols so a
    [128,1] index tile (value e*128+p) gathers W1 for an expert in one
    indirect DMA with 128 8KB descriptors.
  - exp_w2/b1/b2 -> f32 "wallb" [E*128, HT*C+HT+1] gathered the same way.
  - gating emb -> fp16 [V, D], gathered with dma_gather (int16 vocab
    indices, pre-wrapped on host into the [16-partition x replicated]
    layout the Q7 ucode expects).
Expert math: z = relu((tok8.T @ W1_8)/1024 + b1) with fp32 PSUM; mean over
s via the activation accumulator; W2 applied in fp32.

HW gotcha (verified on device): indirect DMA consumes exactly ONE index per
destination partition — multi-index-per-partition gathers return garbage.
"""

import os
import sys

for _p in ("/opt/trn_rl_repo", "/root/.axon_site/_ro/trn_rl_repo"):
    if os.path.isdir(_p) and _p not in sys.path:
        sys.path.insert(0, _p)

import numpy as np

import concourse.bacc as bacc
import concourse.tile as tile
import concourse.mybir as mybir
from concourse.bass import IndirectOffsetOnAxis
from concourse.bass_utils import run_bass_kernel_spmd
from concourse.masks import make_identity

F32 = mybir.dt.float32
F16 = mybir.dt.float16
BF16 = mybir.dt.bfloat16
F8 = mybir.dt.float8e4
I32 = mybir.dt.int32
I16 = mybir.dt.int16
U32 = mybir.dt.uint32

V, D, H, E, C, TOPK = 16000, 1024, 1024, 8, 16, 2
B, S = 64, 512
GATE_H = 256
NCORES = 8
BL = B // NCORES          # samples per core
DT = D // 128             # 8 d-tiles
HT = H // 128             # 8 h-tiles
ST = S // 128             # 4 s-tiles
MT = GATE_H // 128        # 2 gate-hidden tiles
NGRP = 2                  # sample groups per core (pipelining)
GBL = BL // NGRP          # samples per group

ESC = 32.0                # fp8 scale for exp_emb and exp_w1
DESC = 1.0 / (ESC * ESC)  # undo on the way out of PSUM

# f32 wallb columns: W2 (j-major), b1, b2 slot
WB_W2 = 0
WB_B1 = HT * C            # 128
WB_B2 = WB_B1 + HT        # 136
WB_COLS = WB_B2 + 1       # 137

_compiled = {}
last_results = None       # BassKernelResults of the most recent run (for test.py)


def build_program(reps=1):
    """reps>1 repeats the whole compute body (benchmarking aid)."""
    nc = bacc.Bacc("TRN2", target_bir_lowering=False, debug=False, num_devices=NCORES)
    act = mybir.ActivationFunctionType

    x_t = nc.dram_tensor("x_loc", [BL, S], I32, kind="ExternalInput")
    xw_t = nc.dram_tensor("xw16", [128, BL, S // 16], I16, kind="ExternalInput")
    emb_t = nc.dram_tensor("emb16", [V, D], F16, kind="ExternalInput")
    eemb_t = nc.dram_tensor("eemb8", [E * V, D], F8, kind="ExternalInput")
    wall8_t = nc.dram_tensor("wall8", [E * 128, DT * H], F8, kind="ExternalInput")
    wallb_t = nc.dram_tensor("wallb", [E * 128, WB_COLS], F32, kind="ExternalInput")
    gw1_t = nc.dram_tensor("gw1", [D, GATE_H], F32, kind="ExternalInput")
    gb1_t = nc.dram_tensor("gb1", [128, MT], F32, kind="ExternalInput")
    gw2_t = nc.dram_tensor("gw2", [GATE_H, E], F32, kind="ExternalInput")
    gb2_t = nc.dram_tensor("gb2", [E, 1], F32, kind="ExternalInput")
    out_t = nc.dram_tensor("out", [BL, C], F32, kind="ExternalOutput")

    with tile.TileContext(nc) as tc:
        with (
            tc.tile_pool(name="const", bufs=1) as cpool,
            tc.tile_pool(name="dram", bufs=1, space="DRAM") as dpool,
        ):
            # ---- constants ----
            # block identity pair [I|0 ; 0|I] for DoubleRow pair-transposes:
            # out = tok[:,t].T @ idp[:,0] + tok[:,t+1].T @ idp[:,1]
            #     = [tok[:,t].T | tok[:,t+1].T]
            idp = cpool.tile([128, 2, 256], F8)
            nc.gpsimd.memset(idp[:, :, :], 0.0)
            make_identity(nc, idp[:, 0, 0:128], nomemset=True)
            make_identity(nc, idp[:, 1, 128:256], nomemset=True)
            id_f = cpool.tile([128, 128], F32)
            make_identity(nc, id_f[:, :])
            ones_k = cpool.tile([128, 1], F32)      # lhsT for partition-sum MMs
            nc.vector.memset(ones_k[:, :], 1.0)
            ones_m = cpool.tile([1, 128], F32)      # lhsT for K=1 broadcast MMs
            nc.vector.memset(ones_m[:, :], 1.0)
            iota_p = cpool.tile([128, 1], I32)      # value = partition index
            nc.gpsimd.iota(iota_p[:, :], pattern=[[0, 1]], base=0, channel_multiplier=1)

            # token ids, transposed: xt[p, b, t] = x[b, t*128+p]
            xt = cpool.tile([128, BL, ST], I32)
            nc.sync.dma_start(
                out=xt[:, :, :], in_=x_t[:, :].rearrange("b (t p) -> p b t", p=128)
            )
            # int16 wrapped indices for dma_gather (pre-wrapped on host)
            xw = cpool.tile([128, BL, S // 16], I16)
            nc.sync.dma_start(out=xw[:, :, :], in_=xw_t[:, :, :])

            gb1_sb = cpool.tile([128, MT], F32)
            nc.sync.dma_start(out=gb1_sb[:, :], in_=gb1_t[:, :])
            gb2_sb = cpool.tile([E, 1], F32)
            nc.sync.dma_start(out=gb2_sb[:, :], in_=gb2_t[:, :])
            gw1_sb = cpool.tile([128, DT, GATE_H], F32)
            nc.sync.dma_start(
                out=gw1_sb[:, :, :], in_=gw1_t[:, :].rearrange("(j p) g -> p j g", p=128)
            )
            gw2_sb = cpool.tile([128, MT, E], F32)
            nc.sync.dma_start(
                out=gw2_sb[:, :, :], in_=gw2_t[:, :].rearrange("(m p) e -> p m e", p=128)
            )

            consts = dict(
                idp=idp, id_f=id_f, ones_k=ones_k, ones_m=ones_m,
                iota_p=iota_p, xt=xt, xw=xw, gb1_sb=gb1_sb, gb2_sb=gb2_sb,
                gw1_sb=gw1_sb, gw2_sb=gw2_sb,
            )
            tensors = dict(
                emb_t=emb_t, eemb_t=eemb_t, wall8_t=wall8_t, wallb_t=wallb_t,
                out_t=out_t,
            )
            # chain tile serializes reps so the benchmark differential is honest
            chain = None
            if reps > 1:
                chain = cpool.tile([1, 1], F32)
                nc.vector.memset(chain[:, :], 0.0)
            for rep in range(reps):
                _body_once(nc, tc, act, rep, dpool, consts, tensors, chain)

    nc.compile()
    return nc


def _body_once(nc, tc, act, rep, dpool, cn, tn, chain=None):
    sfx = f"_r{rep}"
    idp, id_f = cn["idp"], cn["id_f"]
    ones_k, ones_m, iota_p = cn["ones_k"], cn["ones_m"], cn["iota_p"]
    xt, xw = cn["xt"], cn["xw"]
    gb1_sb, gb2_sb, gw1_sb, gw2_sb = cn["gb1_sb"], cn["gb2_sb"], cn["gw1_sb"], cn["gw2_sb"]
    emb_t, eemb_t = tn["emb_t"], tn["eemb_t"]
    wall8_t, wallb_t, out_t = tn["wall8_t"], tn["wallb_t"], tn["out_t"]

    with (
        tc.tile_pool(name=f"persist{sfx}", bufs=1) as ppool,
        tc.tile_pool(name=f"bc{sfx}", bufs=2) as bcpool,
        # gating pools
        tc.tile_pool(name=f"gat{sfx}", bufs=2) as gpool,
        tc.tile_pool(name=f"gat1{sfx}", bufs=2) as g1pool,
        tc.tile_pool(name=f"gsb{sfx}", bufs=2) as gspool,
        tc.tile_pool(name=f"gps{sfx}", bufs=1, space="PSUM") as gps,
        tc.tile_pool(name=f"gpss{sfx}", bufs=2, space="PSUM") as gps_s,
        # expert pools
        tc.tile_pool(name=f"exi{sfx}", bufs=3) as xipool,
        tc.tile_pool(name=f"etok{sfx}", bufs=2) as tokpool,
        tc.tile_pool(name=f"ew{sfx}", bufs=2) as wpool,
        tc.tile_pool(name=f"ett{sfx}", bufs=2) as ttpool,
        tc.tile_pool(name=f"esm{sfx}", bufs=3) as smpool,
        tc.tile_pool(name=f"ejunk{sfx}", bufs=2) as junkpool,
        tc.tile_pool(name=f"epst{sfx}", bufs=2, space="PSUM") as eps_t,
        tc.tile_pool(name=f"epsz{sfx}", bufs=2, space="PSUM") as eps_z,
        tc.tile_pool(name=f"epso{sfx}", bufs=1, space="PSUM") as eps_o,
    ):
        out_acc = ppool.tile([C, BL], F32)
        nc.vector.memset(out_acc[:, :], 0.0)

        for g in range(NGRP):
            b0 = g * GBL
            # ============ gating for samples [b0, b0+GBL) ============
            # fp16 gather; all arithmetic fp32 so top-2 matches the
            # fp32 reference (margin verified on the seed-0 inputs).
            pooled = gspool.tile([1, GBL * D], F32, tag="pooled")
            for bl in range(GBL):
                b = b0 + bl
                gtok = gpool.tile([128, ST, D], F16, tag="gtok")
                nc.gpsimd.dma_gather(
                    out_ap=gtok[:, :, :],
                    in_ap=emb_t[:, :],
                    idxs_ap=xw[:, b, :],
                    num_idxs=S,
                    num_idxs_reg=S,
                    elem_size=D,
                    transpose=False,
                )
                t01 = g1pool.tile([128, D], F32, tag="t01")
                t23 = g1pool.tile([128, D], F32, tag="t23")
                nc.vector.tensor_add(t01[:, :], gtok[:, 0, :], gtok[:, 1, :])
                nc.vector.tensor_add(t23[:, :], gtok[:, 2, :], gtok[:, 3, :])
                cb = g1pool.tile([128, D], F32, tag="cb")
                nc.vector.tensor_add(cb[:, :], t01[:, :], t23[:, :])
                for h in range(2):
                    pp = gps.tile([1, 512], F32, tag="pool_ps")
                    nc.tensor.matmul(
                        out=pp[:, :],
                        lhsT=ones_k[:, :],
                        rhs=cb[:, h * 512 : (h + 1) * 512],
                        start=True,
                        stop=True,
                    )
                    # mean over S
                    nc.scalar.activation(
                        out=pooled[0:1, bl * D + h * 512 : bl * D + (h + 1) * 512],
                        in_=pp[:, :],
                        func=act.Copy,
                        scale=1.0 / S,
                    )

            # pooled^T [d, b] via K=1 matmuls into one psum tile
            pt_ps = gps_s.tile([128, DT * GBL], F32, tag="gmisc")
            for bl in range(GBL):
                for j in range(DT):
                    nc.tensor.matmul(
                        out=pt_ps[:, j * GBL + bl : j * GBL + bl + 1],
                        lhsT=pooled[0:1, bl * D + j * 128 : bl * D + (j + 1) * 128],
                        rhs=ones_m[0:1, 0:1],
                        start=True,
                        stop=True,
                    )
            pts = gspool.tile([128, DT * GBL], F32, tag="pts")
            nc.vector.tensor_copy(pts[:, :], pt_ps[:, :])

            # gate layer 1 + relu
            hR = gspool.tile([128, MT, GBL], F32, tag="hR")
            for m in range(MT):
                h_ps = gps_s.tile([128, GBL], F32, tag="gmisc")
                for j in range(DT):
                    nc.tensor.matmul(
                        out=h_ps[:, :],
                        lhsT=gw1_sb[:, j, m * 128 : (m + 1) * 128],
                        rhs=pts[:, j * GBL : (j + 1) * GBL],
                        start=(j == 0),
                        stop=(j == DT - 1),
                    )
                nc.scalar.activation(
                    out=hR[:, m, :],
                    in_=h_ps[:, :],
                    func=act.Relu,
                    bias=gb1_sb[:, m : m + 1],
                )

            # gate layer 2 -> logits [e, b]
            l_ps = gps_s.tile([E, GBL], F32, tag="gmisc")
            for m in range(MT):
                nc.tensor.matmul(
                    out=l_ps[:, :],
                    lhsT=gw2_sb[:, m, :],
                    rhs=hR[:, m, :],
                    start=(m == 0),
                    stop=(m == MT - 1),
                )
            l_sb = gspool.tile([E, GBL], F32, tag="l_sb")
            nc.scalar.activation(
                out=l_sb[:, :], in_=l_ps[:, :], func=act.Identity,
                bias=gb2_sb[:, 0:1],
            )
            # transpose logits -> [b, e]
            lt_ps = gps_s.tile([GBL, E], F32, tag="gmisc")
            nc.tensor.matmul(
                out=lt_ps[:, :], lhsT=l_sb[:, :], rhs=id_f[0:E, 0:E],
                start=True, stop=True,
            )
            lt_sb = gspool.tile([GBL, E], F32, tag="lt_sb")
            nc.vector.tensor_copy(lt_sb[:, :], lt_ps[:, :])

            # top-2 of logits == top-2 of softmax (monotone)
            mx = gspool.tile([GBL, 8], F32, tag="mx")
            mi = gspool.tile([GBL, 8], U32, tag="mi")
            nc.vector.max_with_indices(mx[:, :], mi[:, :], lt_sb[:, :])

            # renormalized top-2 softmax weights:
            # rw1 = 1/(1+exp(l2-l1)), rw2 = exp(l2-l1)/(1+exp(l2-l1))
            dlt = gspool.tile([GBL, 1], F32, tag="dlt")
            nc.vector.tensor_sub(dlt[:, :], mx[:, 1:2], mx[:, 0:1])
            q = gspool.tile([GBL, 1], F32, tag="q")
            nc.scalar.activation(out=q[:, :], in_=dlt[:, :], func=act.Exp)
            sden = gspool.tile([GBL, 1], F32, tag="sden")
            nc.vector.tensor_scalar_add(sden[:, :], q[:, :], 1.0)
            rw1 = gspool.tile([GBL, 1], F32, tag="rw1")
            nc.vector.reciprocal(rw1[:, :], sden[:, :])
            rw2 = gspool.tile([GBL, 1], F32, tag="rw2")
            nc.vector.tensor_mul(rw2[:, :], q[:, :], rw1[:, :])

            # pack per-(b,k) scalars: cols bl*8 + {0,1}=e*V, {2,3}=e*128,
            # {6,7}=rw ({4,5} unused)
            ei_f = gspool.tile([GBL, TOPK], F32, tag="ei_f")
            nc.vector.tensor_copy(ei_f[:, :], mi[:, 0:TOPK])
            vals = gspool.tile([GBL, 8], F32, tag="vals")
            nc.vector.tensor_scalar_mul(vals[:, 0:2], ei_f[:, :], float(V))
            nc.vector.tensor_scalar_mul(vals[:, 2:4], ei_f[:, :], 128.0)
            nc.vector.tensor_scalar_mul(vals[:, 4:6], ei_f[:, :], 0.0)
            nc.vector.tensor_copy(vals[:, 6:7], rw1[:, :])
            nc.vector.tensor_copy(vals[:, 7:8], rw2[:, :])

            # broadcast across partitions: bounce through DRAM to get a flat
            # [1, GBL*8] row, then K=1 matmul against ones.
            scratch = dpool.tile([GBL, 8], F32, tag=f"scratch{sfx}_{g}")
            nc.sync.dma_start(out=scratch[:, :], in_=vals[:, :])
            if chain is not None:
                # unused col 4: forces rep r to wait on rep r-1's result
                nc.sync.dma_start(out=scratch[0:1, 4:5], in_=chain[0:1, 0:1])
            flat = gspool.tile([1, GBL * 8], F32, tag="flat")
            nc.sync.dma_start(
                out=flat[0:1, :].rearrange("p (b c) -> p b c", b=GBL),
                in_=scratch[:, :],
            )
            bc_ps = gps_s.tile([128, GBL * 8], F32, tag="gmisc")
            nc.tensor.matmul(
                out=bc_ps[:, :], lhsT=ones_m[:, :], rhs=flat[0:1, :],
                start=True, stop=True,
            )
            BCf = bcpool.tile([128, GBL * 8], F32, tag="bcf")
            BCi = bcpool.tile([128, GBL * 8], I32, tag="bci")
            nc.vector.tensor_copy(BCf[:, :], bc_ps[:, :])
            nc.vector.tensor_copy(BCi[:, :], bc_ps[:, :])  # cast f32->i32

            # ============ experts for this group (fp8) ============
            for bl in range(GBL):
                b = b0 + bl
                for k in range(TOPK):
                    cEV = bl * 8 + k
                    cE128 = bl * 8 + 2 + k
                    cRW = bl * 8 + 6 + k

                    tok_idx = xipool.tile([128, ST], I32, tag="tok_idx")
                    nc.vector.tensor_add(
                        tok_idx[:, :],
                        xt[:, b, :],
                        BCi[:, cEV : cEV + 1].to_broadcast([128, ST]),
                    )
                    w_idx = xipool.tile([128, 1], I32, tag="w_idx")
                    nc.vector.tensor_add(
                        w_idx[:, :], iota_p[:, :], BCi[:, cE128 : cE128 + 1]
                    )

                    tok8 = tokpool.tile([128, ST, D], F8, tag="tok8")
                    for t in range(ST):
                        nc.gpsimd.indirect_dma_start(
                            out=tok8[:, t, :],
                            out_offset=None,
                            in_=eemb_t[:, :],
                            in_offset=IndirectOffsetOnAxis(
                                ap=tok_idx[:, t : t + 1], axis=0
                            ),
                        )
                    # W1 (fp8, scaled) and W2/b1/b2 (f32) for this expert
                    wg8 = wpool.tile([128, DT, H], F8, tag="wg8")
                    nc.gpsimd.indirect_dma_start(
                        out=wg8[:, :, :].rearrange("p t h -> p (t h)"),
                        out_offset=None,
                        in_=wall8_t[:, :],
                        in_offset=IndirectOffsetOnAxis(ap=w_idx[:, :], axis=0),
                    )
                    wgb = wpool.tile([128, WB_COLS], F32, tag="wgb")
                    nc.gpsimd.indirect_dma_start(
                        out=wgb[:, :],
                        out_offset=None,
                        in_=wallb_t[:, :],
                        in_offset=IndirectOffsetOnAxis(ap=w_idx[:, :], axis=0),
                    )

                    # transpose tok8 -> tokT8[d, s]: DoubleRow pair-transpose
                    # (two 128x128 tiles per matmul against the block
                    # identity pair), then DVE f32->fp8 copies out of PSUM
                    tokT8 = ttpool.tile([128, DT, S], F8, tag="tokT8")
                    for j in range(DT):
                        tp = eps_t.tile([128, S], F32, tag="tp")
                        for t in range(0, ST, 2):
                            nc.tensor.matmul(
                                out=tp[:, t * 128 : (t + 2) * 128],
                                lhsT=tok8[:, t : t + 2, j * 128 : (j + 1) * 128],
                                rhs=idp[:, :, :],
                                start=True,
                                stop=True,
                                perf_mode=mybir.MatmulPerfMode.DoubleRow,
                            )
                        nc.vector.tensor_copy(tokT8[:, j, :], tp[:, :])

                    # z[h_tile] = relu((tokT8.T @ W1_8)/ESC^2 + b1);
                    # DoubleRow: 2 k-subtiles per matmul. accumulate sum over s.
                    pacc = smpool.tile([128, HT], F32, tag="pacc")
                    for j2 in range(HT):
                        z_ps = eps_z.tile([128, S], F32, tag="z")
                        for t in range(0, DT, 2):
                            nc.tensor.matmul(
                                out=z_ps[:, :],
                                lhsT=wg8[:, t : t + 2, j2 * 128 : (j2 + 1) * 128],
                                rhs=tokT8[:, t : t + 2, :],
                                start=(t == 0),
                                stop=(t == DT - 2),
                                perf_mode=mybir.MatmulPerfMode.DoubleRow,
                            )
                        zjunk = junkpool.tile([128, S], BF16, tag="zjunk")
                        nc.scalar.activation(
                            out=zjunk[:, :],
                            in_=z_ps[:, :],
                            func=act.Relu,
                            bias=wgb[:, WB_B1 + j2 : WB_B1 + j2 + 1],
                            scale=DESC,
                            accum_out=pacc[:, j2 : j2 + 1],
                        )

                    psc = smpool.tile([128, HT], F32, tag="psc")
                    nc.vector.tensor_scalar_mul(psc[:, :], pacc[:, :], 1.0 / S)

                    eo_ps = eps_o.tile([C, 1], F32, tag="eo")
                    for j2 in range(HT):
                        nc.tensor.matmul(
                            out=eo_ps[:, :],
                            lhsT=wgb[:, j2 * C : (j2 + 1) * C],
                            rhs=psc[:, j2 : j2 + 1],
                            start=(j2 == 0),
                            stop=(j2 == HT - 1),
                        )
                    eo1 = smpool.tile([C, 1], F32, tag="eo1")
                    nc.scalar.activation(
                        out=eo1[:, :], in_=eo_ps[:, :], func=act.Identity,
                        bias=wgb[0:C, WB_B2 : WB_B2 + 1],
                    )
                    eo2 = smpool.tile([C, 1], F32, tag="eo2")
                    nc.vector.tensor_mul(eo2[:, :], eo1[:, :], BCf[0:C, cRW : cRW + 1])
                    nc.vector.tensor_add(
                        out_acc[:, b : b + 1], out_acc[:, b : b + 1], eo2[:, :]
                    )

        if chain is not None:
            nc.vector.tensor_copy(chain[0:1, 0:1], out_acc[0:1, 0:1])
        nc.sync.dma_start(
            out=out_t[:, :].rearrange("b c -> c b"), in_=out_acc[:, :]
        )


def _prep_inputs(inputs):
    """Host-side dtype casts + re-layouts shared by all cores."""
    import ml_dtypes

    f32 = np.float32
    f16 = np.float16
    fp8 = ml_dtypes.float8_e4m3

    x = np.asarray(inputs["x"]).astype(np.int32)
    # int16 indices wrapped for dma_gather: xw16[16g+p, b, c] = x[b, c*16+p]
    xw = x.reshape(B, S // 16, 16).transpose(2, 0, 1).astype(np.int16)  # [16, B, 32]
    xw16 = np.tile(xw, (8, 1, 1))                                       # [128, B, 32]

    emb16 = np.ascontiguousarray(np.asarray(inputs["emb"], dtype=f32).astype(f16))
    eemb8 = np.ascontiguousarray(
        (np.asarray(inputs["exp_emb"], dtype=f32) * ESC).reshape(E * V, D)
    ).astype(fp8)

    w1 = np.asarray(inputs["exp_w1"], dtype=f32)          # [E, D, H]
    ew1 = w1.reshape(E, DT, 128, H).transpose(0, 2, 1, 3).reshape(E * 128, DT * H)
    wall8 = np.ascontiguousarray(ew1 * ESC).astype(fp8)

    w2 = np.asarray(inputs["exp_w2"], dtype=f32)          # [E, H, C]
    ew2 = w2.reshape(E, HT, 128, C).transpose(0, 2, 1, 3).reshape(E * 128, HT * C)
    b1 = np.asarray(inputs["exp_b1"], dtype=f32)          # [E, H]
    b1r = b1.reshape(E, HT, 128).transpose(0, 2, 1).reshape(E * 128, HT)
    b2 = np.asarray(inputs["exp_b2"], dtype=f32)          # [E, C]
    b2slot = np.zeros((E * 128, 1), f32)
    for e in range(E):
        b2slot[e * 128 : e * 128 + C, 0] = b2[e]
    wallb = np.zeros((E * 128, WB_COLS), f32)
    wallb[:, WB_W2 : WB_W2 + HT * C] = ew2
    wallb[:, WB_B1 : WB_B1 + HT] = b1r
    wallb[:, WB_B2 : WB_B2 + 1] = b2slot
    wallb = np.ascontiguousarray(wallb)

    gw1 = np.ascontiguousarray(np.asarray(inputs["gate_w1"], dtype=f32))
    gb1 = np.ascontiguousarray(
        np.asarray(inputs["gate_b1"], dtype=f32).reshape(MT, 128).T
    )
    gw2 = np.ascontiguousarray(np.asarray(inputs["gate_w2"], dtype=f32))
    gb2 = np.ascontiguousarray(np.asarray(inputs["gate_b2"], dtype=f32).reshape(E, 1))

    shared = dict(
        emb16=emb16, eemb8=eemb8, wall8=wall8, wallb=wallb,
        gw1=gw1, gb1=gb1, gw2=gw2, gb2=gb2,
    )
    return x, xw16, shared


def kernel(**inputs) -> np.ndarray:
    global last_results
    if "nc" not in _compiled:
        _compiled["nc"] = build_program()
    nc = _compiled["nc"]

    x, xw16, shared = _prep_inputs(inputs)
    in_maps = [
        {
            "x_loc": np.ascontiguousarray(x[c * BL : (c + 1) * BL]),
            "xw16": np.ascontiguousarray(xw16[:, c * BL : (c + 1) * BL]),
            **shared,
        }
        for c in range(NCORES)
    ]
    res = run_bass_kernel_spmd(nc, in_maps, list(range(NCORES)))
    last_results = res
    out = np.concatenate([res.results[c]["out"] for c in range(NCORES)], axis=0)
    return np.ascontiguousarray(out.astype(np.float32))
